# revision 2
# baseline (speedup 1.0000x reference)
"""Trainium2 Bass kernel for nn_Copy_56470230008202 (sparse_attention), v4.

Fully sequence-sharded, ZERO collectives:
  Core i owns query positions t in [256i, 256i+256). The reference's
  view-scramble maps output row l' = n*128 + pg to query positions
  t = pg*16 + j, so rows owned by core i are {n*128 + pg : pg//16 == i} --
  every stage (conv0, conv1, scores, softmax, mix, scramble, out-proj, V/C
  logits) is local to the core's t-slice. The host reassembles the row
  permutation and adds the V/C bias.

  - conv0/conv1: all 1024 channels over local 256 positions (+host-computed
    halo columns of x0 for the conv zero-pad edges).
  - attention: all 16 heads x [2048 s, 256 t] per core (same FLOPs as
    head-sharding); kvT/kvag replicated (8.4 MB SBUF).
  - softmax denominator via an appended ones-column in kvag; no max
    subtraction (scores in [-6,6]).
  - out-proj: [1024 out-ch, 256 local l'-cols]; V/C logits vs full VC^T
    streamed in 520 KB blocks, psum evacuated bf16.
  All matmuls bf16 / fp32 PSUM. Weight-norm, selu(f), packing on host.
"""

import os
import sys

for _p in ("/opt/trn_rl_repo", "/root/.axon_site/_ro/trn_rl_repo"):
    if os.path.isdir(_p) and _p not in sys.path:
        sys.path.append(_p)

import numpy as np
import ml_dtypes

import concourse.bass as bass
import concourse.mybir as mybir
from concourse import bacc
from concourse.tile import TileContext
from concourse.bass_utils import run_bass_kernel_spmd

F32 = mybir.dt.float32
BF16 = mybir.dt.bfloat16
ALU = mybir.AluOpType
ACTF = mybir.ActivationFunctionType

H, NH, HD = 1024, 16, 64
CIN, VOCAB, LIMIT, L, S = 1280, 32000, 512, 2048, 2048
VC = VOCAB + LIMIT              # 32512
VBW = 508
NG, GW = 16, 2032
NCORES = 8
TSL = L // NCORES               # 256
LAM, ALPHA = 1.0507009873554805, 1.6732632423543772


def _selu_from_psum(nc, tmp, psum_ap, bias_ap, out_ap, P, N, idx, zeros=None):
    """out = selu(z) given psum = LAM*z (lambda folded into weights+bias)."""
    m = tmp.tile([P, N], F32, name=f"selu_m{idx}", tag=f"selu_m{P}x{N}")
    r = tmp.tile([P, N], F32, name=f"selu_r{idx}", tag=f"selu_r{P}x{N}")
    e = tmp.tile([P, N], F32, name=f"selu_e{idx}", tag=f"selu_e{P}x{N}")
    t = tmp.tile([P, N], F32, name=f"selu_t{idx}", tag=f"selu_t{P}x{N}")
    z = zeros[0:P, :N]
    nc.vector.scalar_tensor_tensor(m, psum_ap, bias_ap, z, op0=ALU.add, op1=ALU.min)
    nc.vector.scalar_tensor_tensor(r, psum_ap, bias_ap, z, op0=ALU.add, op1=ALU.max)
    nc.scalar.activation(e, m, ACTF.Exp, scale=1.0 / LAM)
    nc.vector.tensor_scalar(t, e, LAM * ALPHA, -LAM * ALPHA, op0=ALU.mult, op1=ALU.add)
    nc.vector.tensor_tensor(out_ap, t, r, op=ALU.add)


def build_program():
    nc = bacc.Bacc("TRN2", target_bir_lowering=False, debug=False,
                   num_devices=NCORES)
    # per-core
    oTp = nc.declare_dram_parameter("oTp", [128, 10 * 258], BF16, isOutput=False)
    x0h = nc.declare_dram_parameter("x0h", [128, 16], BF16, isOutput=False)
    # replicated
    w0p = nc.declare_dram_parameter("w0p", [128, 240 * 128], BF16, isOutput=False)
    w1p = nc.declare_dram_parameter("w1p", [128, 192 * 128], BF16, isOutput=False)
    kvp = nc.declare_dram_parameter("kvp", [128, 16 * S], BF16, isOutput=False)
    kvagp = nc.declare_dram_parameter("kvagp", [128, 256 * 128], BF16, isOutput=False)
    wop = nc.declare_dram_parameter("wop", [128, 16 * 1024], BF16, isOutput=False)
    cst = nc.declare_dram_parameter("cst", [128, 24], F32, isOutput=False)
    vctp = nc.declare_dram_parameter("vctp", [NG, 8, 128, GW], BF16, isOutput=False)
    out = nc.declare_dram_parameter("out", [2, NG, 128, GW], BF16, isOutput=True)

    with TileContext(nc) as tc:
        _emit(tc, oTp, x0h, w0p, w1p, kvp, kvagp, wop, cst, vctp, out)
    if not nc.is_finalized():
        nc.finalize()
    return nc


def _emit(tc, oTp, x0h, w0p, w1p, kvp, kvagp, wop, cst, vctp, out):
    nc = tc.nc

    with tc.tile_pool(name="const", bufs=1) as constp, \
         tc.tile_pool(name="persist", bufs=1) as pers:
        zeros = constp.tile([128, 512], F32)
        nc.vector.memset(zeros, 0.0)
        cst_sb = constp.tile([128, 24], F32)
        nc.scalar.dma_start(out=cst_sb, in_=cst[:, :])

        cat = pers.tile([128, 16 * 256], BF16)    # col = k*256 + n*16 + pg
        aoT = pers.tile([128, 8 * 256], BF16)     # col = m*256 + h*128 + r
        q_loc = pers.tile([128, 8 * 256], BF16)   # ch co*128+p at col co*256+t
        kvT0_sb = pers.tile([128, S], BF16)       # head 0 kv (early stream)
        kvag0_sb = pers.tile([128, 16 * 128], BF16)

        # ---------------- conv0 + conv1 (local) ----------------
        with tc.tile_pool(name="c0", bufs=1) as c0p, \
             tc.tile_pool(name="c1", bufs=1) as c1p, \
             tc.tile_pool(name="c0ps", bufs=3, space="PSUM") as c0ps, \
             tc.tile_pool(name="c1ps", bufs=3, space="PSUM") as c1ps, \
             tc.tile_pool(name="ctmp", bufs=2) as ctmp:
            oT_sb = c0p.tile([128, 10 * 258], BF16)
            w0_sb = c0p.tile([128, 240 * 128], BF16)
            # first-need order: w0 chunk0 k-part0 + first oT slices gate MM0
            nc.sync.dma_start(out=w0_sb[:, 0:1280], in_=w0p[:, 0:1280])
            for ci in range(10):
                nc.sync.dma_start(out=oT_sb[:, ci * 258:(ci + 1) * 258],
                                  in_=oTp[:, ci * 258:(ci + 1) * 258])
            nc.sync.dma_start(out=w0_sb[:, 1280:3840], in_=w0p[:, 1280:3840])
            for co in range(1, 8):
                nc.sync.dma_start(out=w0_sb[:, co * 3840:(co + 1) * 3840],
                                  in_=w0p[:, co * 3840:(co + 1) * 3840])
            # head 0's kv streams before w1 so attention starts unstalled
            nc.sync.dma_start(out=kvT0_sb, in_=kvp[:, 0:S])
            nc.sync.dma_start(out=kvag0_sb, in_=kvagp[:, 0:2048])
            w1_sb = c1p.tile([128, 192 * 128], BF16)
            for co in range(8):
                nc.sync.dma_start(out=w1_sb[:, co * 3072:(co + 1) * 3072],
                                  in_=w1p[:, co * 3072:(co + 1) * 3072])

            # x0 chunk cols m in [0,258) <-> t = T0 - 1 + m
            x0_sb = c1p.tile([128, 8 * 258], BF16)
            x0re = x0_sb.rearrange("p (c w) -> p c w", w=258)
            nc.scalar.dma_start(out=x0re[:, :, 0:1], in_=x0h[:, 0::2])
            nc.scalar.dma_start(out=x0re[:, :, 257:258], in_=x0h[:, 1::2])

            for co in range(8):
                ps = c0ps.tile([128, 256], F32, name="c0psum", tag="c0psum")
                idx = 0
                for k in range(3):
                    for ci in range(10):
                        u = co * 30 + k * 10 + ci
                        nc.tensor.matmul(
                            ps, lhsT=w0_sb[:, u * 128:(u + 1) * 128],
                            rhs=oT_sb[:, ci * 258 + k: ci * 258 + k + 256],
                            start=(idx == 0), stop=(idx == 29))
                        idx += 1
                _selu_from_psum(nc, ctmp, ps, cst_sb[:, co:co + 1],
                                x0_sb[:, co * 258 + 1: co * 258 + 257],
                                128, 256, f"c0_{co}", zeros=zeros)

            for co in range(8):
                ps = c1ps.tile([128, 256], F32, name="c1psum", tag="c1psum")
                idx = 0
                for k in range(3):
                    for ci in range(8):
                        u = co * 24 + k * 8 + ci
                        nc.tensor.matmul(
                            ps, lhsT=w1_sb[:, u * 128:(u + 1) * 128],
                            rhs=x0_sb[:, ci * 258 + k: ci * 258 + k + 256],
                            start=(idx == 0), stop=(idx == 23))
                        idx += 1
                _selu_from_psum(nc, ctmp, ps, cst_sb[:, 8 + co:9 + co],
                                q_loc[:, co * 256:(co + 1) * 256],
                                128, 256, f"c1_{co}", zeros=zeros)

        # ---------------- attention (all 16 heads, local t) ----------------
        with tc.tile_pool(name="wo", bufs=1) as wop_:
            catre = cat.rearrange("p (k r) -> p k r", r=256)
            with tc.tile_pool(name="kvpool", bufs=1) as kvq, \
                 tc.tile_pool(name="attn", bufs=1) as atp, \
                 tc.tile_pool(name="ppool", bufs=8) as ppool, \
                 tc.tile_pool(name="scps", bufs=2, space="PSUM") as scps, \
                 tc.tile_pool(name="mixps", bufs=4, space="PSUM") as mixps:
                # padded kv: per head [128, 2048] kvT (other half zero) and
                # 16 [128, 128] kvag tiles (64 kv | ones | 63 zero). Streamed
                # on the sync ring after the conv weights, per-head order.
                kvT_sb = kvq.tile([128, 15 * S], BF16)
                kvag_sb = kvq.tile([128, 240 * 128], BF16)
                for n in range(1, NH):
                    nc.sync.dma_start(
                        out=kvT_sb[:, (n - 1) * S:n * S],
                        in_=kvp[:, n * S:(n + 1) * S])
                    nc.sync.dma_start(
                        out=kvag_sb[:, (n - 1) * 2048:n * 2048],
                        in_=kvagp[:, n * 2048:(n + 1) * 2048])
                wo_sb = wop_.tile([128, 16 * 1024], BF16)
                nc.sync.dma_start(out=wo_sb, in_=wop[:, :])
                # catq copies: cat[(8+k)*256 + n*16 + pg, part jj*64+d]
                #   <- q_loc[(n%2)*64+d, (n//2)*256 + pg*16 + 2k+jj]
                for n in range(NH):
                    qre = q_loc[(n % 2) * 64:(n % 2) * 64 + 64,
                                (n // 2) * 256:(n // 2 + 1) * 256] \
                        .rearrange("p (pg j) -> p j pg", j=16)
                    for jj in range(2):
                        nc.vector.tensor_copy(
                            out=catre[jj * 64:(jj + 1) * 64, 8:16,
                                      n * 16:n * 16 + 16],
                            in_=qre[:, jj::2, :])

                def emit_scores(n):
                    # 4 psum chunks of [128, 1024]; chunk c holds s-tiles
                    # 4c..4c+3 (cols sb*256 + t)
                    ptiles = []
                    for c in range(4):
                        ps2 = scps.tile([128, 1024], F32, name="ps_sc",
                                        tag="ps_sc")
                        for sb in range(4):
                            st = 4 * c + sb
                            nc.tensor.matmul(
                                ps2[:, sb * 256:(sb + 1) * 256],
                                lhsT=(kvT0_sb if n == 0 else kvT_sb)[
                                    :, (n - 1 if n else 0) * 2048 + st * 128:
                                    (n - 1 if n else 0) * 2048 + (st + 1) * 128],
                                rhs=q_loc[:, (n // 2) * 256:
                                          (n // 2 + 1) * 256],
                                start=True, stop=True)
                        p2 = ppool.tile([128, 1024], BF16, name="p_t", tag="p")
                        nc.scalar.activation(p2, ps2, ACTF.Exp, scale=0.125)
                        ptiles.append(p2)
                    return ptiles

                def emit_mix(n, ptiles):
                    ps_mix = mixps.tile([128, 256], F32, name="ps_mix",
                                        tag="ps_mix")
                    for st in range(16):
                        nc.tensor.matmul(
                            ps_mix,
                            lhsT=(kvag0_sb if n == 0 else kvag_sb)[
                                :, (((n - 1) if n else 0) * 16 + st) * 128:
                                (((n - 1) if n else 0) * 16 + st + 1) * 128],
                            rhs=ptiles[st // 4][:, (st % 4) * 256:
                                                (st % 4 + 1) * 256],
                            start=(st == 0), stop=(st == 15))
                    recip = atp.tile([1, 256], F32, name="recip", tag="recip",
                                     bufs=4)
                    nc.vector.reciprocal(recip, ps_mix[64:65, :])
                    bc = atp.tile([64, 256], F32, name="bc", tag="bc", bufs=4)
                    nc.gpsimd.partition_broadcast(bc, recip)
                    mre = ps_mix[0:64, :].rearrange("p (pg j) -> p j pg", j=16)
                    bre = bc.rearrange("p (pg j) -> p j pg", j=16)
                    for jj in range(2):
                        nc.vector.tensor_tensor(
                            out=catre[jj * 64:(jj + 1) * 64, 0:8,
                                      n * 16:n * 16 + 16],
                            in0=mre[:, jj::2, :],
                            in1=bre[:, jj::2, :],
                            op=ALU.mult)

                pts = {0: emit_scores(0)}
                for n in range(NH):
                    if n + 1 < NH:
                        pts[n + 1] = emit_scores(n + 1)
                    emit_mix(n, pts.pop(n))

            # ------------- out-projection -------------
            with tc.tile_pool(name="ops", bufs=2, space="PSUM") as ops, \
                 tc.tile_pool(name="otmp", bufs=2) as otmp:
                for m in range(8):
                    ps_o = ops.tile([128, 256], F32, name="ps_o", tag="ps_o")
                    for k in range(16):
                        nc.tensor.matmul(
                            ps_o,
                            lhsT=wo_sb[:, k * 1024 + m * 128:
                                       k * 1024 + (m + 1) * 128],
                            rhs=cat[:, k * 256:(k + 1) * 256],
                            start=(k == 0), stop=(k == 15))
                    _selu_from_psum(nc, otmp, ps_o, cst_sb[:, 16 + m:17 + m],
                                    aoT[:, m * 256:(m + 1) * 256],
                                    128, 256, f"o_{m}", zeros=zeros)

        # ------------- V/C logits (streamed) -------------
        with tc.tile_pool(name="vstream", bufs=28) as vsp, \
             tc.tile_pool(name="vstage", bufs=6) as vst, \
             tc.tile_pool(name="vps", bufs=3, space="PSUM") as vps:
            for g in range(NG):
                vtiles = []
                for k in range(8):
                    vt = vsp.tile([128, GW], BF16, name="vt", tag="vct")
                    nc.sync.dma_start(out=vt, in_=vctp[g, k, :, :])
                    vtiles.append(vt)
                for h in range(2):
                    stg = vst.tile([128, GW], BF16, name="vstage", tag="vstage")
                    for up in range(2):
                        ps2 = vps.tile([128, 1024], F32, name="ps_v", tag="ps_v")
                        for u2 in range(2):
                            u = up * 2 + u2
                            for k in range(8):
                                nc.tensor.matmul(
                                    ps2[:, u2 * 512: u2 * 512 + VBW],
                                    lhsT=aoT[:, k * 256 + h * 128:
                                             k * 256 + h * 128 + 128],
                                    rhs=vtiles[k][:, u * VBW:(u + 1) * VBW],
                                    start=(k == 0), stop=(k == 7))
                        src = ps2.rearrange("p (u w) -> p u w", w=512)[:, :, 0:VBW]
                        dst = stg.rearrange("p (u w) -> p u w", w=VBW)[:, up * 2:up * 2 + 2, :]
                        nc.vector.tensor_copy(out=dst, in_=src)
                    nc.gpsimd.dma_start(out=out[h, g, :, :], in_=stg)


# ---------------- host side ----------------

def _wn_conv(v, g):
    n = np.sqrt((v * v).sum(axis=(1, 2), keepdims=True))
    return g[:, None, None] * v / n


def _wn_lin(v, g):
    return g[:, None] * v / np.linalg.norm(v, axis=1, keepdims=True)


def _selu_np(x):
    return np.where(x > 0, LAM * x,
                    LAM * ALPHA * (np.exp(np.minimum(x, 0)) - 1)).astype(np.float32)


def _bf16(x):
    return np.ascontiguousarray(x.astype(ml_dtypes.bfloat16))


def _f32(x):
    return np.ascontiguousarray(x.astype(np.float32))


_PROGRAM_CACHE = {}


def kernel(o, f, q0_v, q0_g, q0_b, q1_v, q1_g, q1_b,
           out_v, out_g, out_b, V_v, V_g, V_b, C_v, C_g, C_b):
    o, f = np.asarray(o, np.float32), np.asarray(f, np.float32)

    w0 = _wn_conv(np.asarray(q0_v), np.asarray(q0_g))            # (H, CIN, 3)
    w1 = _wn_conv(np.asarray(q1_v), np.asarray(q1_g))            # (H, H, 3)
    b0 = np.asarray(q0_b, np.float32)
    b1 = np.asarray(q1_b, np.float32)
    woutT = np.ascontiguousarray(_wn_lin(np.asarray(out_v), np.asarray(out_g)).T) * LAM
    outb_l = np.asarray(out_b) * LAM
    vc = np.concatenate([_wn_lin(np.asarray(V_v), np.asarray(V_g)),
                         _wn_lin(np.asarray(C_v), np.asarray(C_g))], axis=0)
    vct = np.ascontiguousarray(vc.T)                             # (H, 32512)
    kv = _selu_np(f)                                             # (S, H)

    W0 = (w0 * LAM).transpose(1, 0, 2).reshape(10, 128, 8, 128, 3)
    w0pk = _bf16(W0.transpose(1, 2, 4, 0, 3).reshape(128, 240 * 128))
    W1 = (w1 * LAM).transpose(1, 0, 2).reshape(8, 128, 8, 128, 3)
    w1pk = _bf16(W1.transpose(1, 2, 4, 0, 3).reshape(128, 192 * 128))
    wopk = _bf16(woutT.reshape(16, 128, 1024).transpose(1, 0, 2)
                 .reshape(128, 16 * 1024))
    vctp = _bf16(vct.reshape(8, 128, NG, GW).transpose(2, 0, 1, 3))
    csti = np.zeros((128, 24), np.float32)
    csti[:, 0:8] = (b0 * LAM).reshape(8, 128).T
    csti[:, 8:16] = (b1 * LAM).reshape(8, 128).T
    csti[:, 16:24] = outb_l.reshape(8, 128).T
    csti = _f32(csti)

    # kvT padded: head n block [128, 2048] with kv dims at partitions
    # (n%2)*64+d and zeros elsewhere (full-128 stationary -> FWL).
    kvT_full = np.ascontiguousarray(kv.T)                        # (H, S)
    kvtp = np.zeros((16, 128, S), np.float32)
    for n in range(NH):
        kvtp[n, (n % 2) * 64:(n % 2) * 64 + 64, :] = \
            kvT_full[n * 64:(n + 1) * 64, :]
    kvpk = _bf16(kvtp.transpose(1, 0, 2).reshape(128, 16 * S))
    # kvag padded: (n, st) tile [128 s, 128]: cols 0-63 kv, 64 ones, rest 0
    kvag_all = np.zeros((16, 16, 128, 128), np.float32)
    for n in range(NH):
        kvag_all[n, :, :, 0:64] = kv[:, n * 64:(n + 1) * 64] \
            .reshape(16, 128, 64)
        kvag_all[n, :, :, 64] = 1.0
    kvagpk = _bf16(kvag_all.transpose(2, 0, 1, 3).reshape(128, 256 * 128))

    # x0 halo columns
    oT_pad = np.zeros((CIN, L + 4), np.float32)
    oT_pad[:, 2:L + 2] = o.T                                     # col j <-> t=j-2
    halos = np.zeros((NCORES, 2, H), np.float32)
    for i in range(NCORES):
        for side, t in ((0, 256 * i - 1), (1, 256 * i + 256)):
            if 0 <= t < L:
                win = oT_pad[:, t + 1:t + 4]
                z = np.einsum('hck,ck->h', w0.astype(np.float32), win) + b0
                halos[i, side] = _selu_np(z)

    if "nc" not in _PROGRAM_CACHE:
        _PROGRAM_CACHE["nc"] = build_program()
    nc = _PROGRAM_CACHE["nc"]

    in_maps = []
    for i in range(NCORES):
        T0 = 256 * i
        oT_sl = oT_pad[:, T0 + 1: T0 + 259]                      # (CIN, 258)
        in_maps.append({
            "oTp": _bf16(oT_sl.reshape(10, 128, 258).transpose(1, 0, 2)
                         .reshape(128, 10 * 258)),
            "x0h": _bf16(halos[i].T.reshape(8, 128, 2).transpose(1, 0, 2)
                         .reshape(128, 16)),
            "w0p": w0pk,
            "w1p": w1pk,
            "kvp": kvpk,
            "kvagp": kvagpk,
            "wop": wopk,
            "cst": csti,
            "vctp": vctp,
        })

    kwargs = {}
    if os.environ.get("NN_COPY_TRACE", "0") == "1":
        kwargs = dict(trace=True)
    res = run_bass_kernel_spmd(nc, in_maps, core_ids=list(range(NCORES)), **kwargs)
    global LAST_RESULTS
    LAST_RESULTS = res
    # reassemble: core i, psum partition r of half h, group g covers local
    # col c = h*128 + r -> global row l' = (c//16)*128 + 16*i + c%16
    full = np.empty((L, VC), np.float32)
    c = np.arange(256)
    for i in range(NCORES):
        od = np.asarray(res.results[i]["out"]).astype(np.float32)  # (2,NG,128,GW)
        block = od.transpose(0, 2, 1, 3).reshape(256, VC)
        rows = (c // 16) * 128 + 16 * i + (c % 16)
        full[rows] = block
    full += np.concatenate([np.asarray(V_b), np.asarray(C_b)])[None, :]
    return full
